# revision 19
# baseline (speedup 1.0000x reference)
"""Trainium2 Bass kernel for NearestNeighborMatcher (retrieval_knn).

Contract: kernel(**inputs) takes FULL inputs (B=8 batches), shards one batch
element per NeuronCore (8 cores, data-parallel, no collectives), and returns
the FULL output tuple (matches0, matches1, mscores0, mscores1, sim).

Device computes per core: normalized bf16 descriptor transposes, the two
4096x4096 similarity matmuls (sim and simT), fp32 sim written to HBM, and
per-row top-8 candidates of every 2048-wide half via the DVE MAX8 unit for
both directions. Host finishes with O(n) work: top-2 merge, ratio/distance
thresholds, argmax lookup for the (rare) mask-passing rows from the sim
output itself, and the mutual check.
"""

import sys

sys.path.insert(0, "/opt/trn_rl_repo")

import numpy as np

B, N, M, D = 8, 4096, 4096, 128
RATIO_THRESH = 0.8
DIST_THRESH = 0.7
EPS = 1e-12
HALF = 2048  # PSUM accumulation / evacuation granularity

_CACHE = {}


def _build(n=N, m=M, d=D):
    import concourse.bass as bass
    import concourse.mybir as mybir
    from concourse import bacc, tile
    from concourse.bass import ts
    from concourse.masks import make_identity

    f32 = mybir.dt.float32
    bf16 = mybir.dt.bfloat16
    Alu = mybir.AluOpType
    Act = mybir.ActivationFunctionType

    assert d == 128 and n % 128 == 0 and m % 128 == 0
    RTN, RTM = n // 128, m // 128
    half = min(HALF, m)
    assert n % half == 0 and m % half == 0
    NCH_N, NCH_M = m // half, n // half  # halves per row, per direction

    nc = bacc.Bacc(
        "TRN2", target_bir_lowering=False, debug=False, enable_asserts=True
    )
    d0 = nc.dram_tensor("descriptors0", [n, d], f32, kind="ExternalInput")
    d1 = nc.dram_tensor("descriptors1", [m, d], f32, kind="ExternalInput")
    sim_o = nc.dram_tensor("sim", [n, m], f32, kind="ExternalOutput")
    th0 = nc.dram_tensor("top8h0", [1, n * NCH_N * 8], f32, kind="ExternalOutput")
    th1 = nc.dram_tensor("top8h1", [1, m * NCH_M * 8], f32, kind="ExternalOutput")

    with tile.TileContext(nc) as tc:
        import contextlib

        ctx = contextlib.ExitStack()
        with ctx:
            persist = ctx.enter_context(tc.tile_pool(name="persist", bufs=1))
            ldpool = ctx.enter_context(tc.tile_pool(name="ld", bufs=RTN + RTM))
            sqpool = ctx.enter_context(tc.tile_pool(name="sq", bufs=2))
            psum = ctx.enter_context(
                tc.tile_pool(name="psum", bufs=2, space="PSUM")
            )
            simpool = ctx.enter_context(tc.tile_pool(name="simbuf", bufs=4))

            idt = persist.tile([128, 128], bf16, tag="idt")
            make_identity(nc, idt[:])

            d0T = persist.tile([128, n], bf16, tag="d0T")
            d1T = persist.tile([128, m], bf16, tag="d1T")

            # ---------------- prologue: normalize + transpose ----------------
            # Split into per-half-tensor pieces so the main loop's first
            # quadrant can start after only the first halves are built.
            def pro_squares(src, t0, t1_, tag):
                lds, ss = _state[tag]
                on_act = tag == "0"
                for t in range(t0, t1_):
                    ld = ldpool.tile([128, 128], f32, tag="ld")
                    nc.sync.dma_start(ld[:], src.ap()[ts(t, 128), :])
                    lds[t] = ld
                    sq = sqpool.tile([128, 128], f32, tag="sq")
                    if on_act:
                        nc.scalar.activation(
                            sq[:], ld[:], Act.Square, accum_out=ss[:, t : t + 1]
                        )
                    else:
                        nc.vector.scalar_tensor_tensor(
                            sq[:],
                            ld[:],
                            0.0,
                            ld[:],
                            op0=Alu.add,
                            op1=Alu.mult,
                            accum_out=ss[:, t : t + 1],
                        )

            def pro_norm(t0, t1_, tag):
                # z = 1/max(sqrt(ss), eps) with one Newton step on rsqrt
                lds, ss = _state[tag]
                w = t1_ - t0
                sl = slice(t0, t1_)
                sroot = sqpool.tile([128, w], f32, tag="sr")
                nc.scalar.activation(sroot[:], ss[:, sl], Act.Sqrt)
                nc.vector.tensor_scalar_max(sroot[:], sroot[:], float(EPS))
                z = _z[tag]
                nc.vector.reciprocal(z[:, sl], sroot[:])
                t1t = sqpool.tile([128, w], f32, tag="nt1")
                nc.vector.tensor_mul(t1t[:], z[:, sl], z[:, sl])
                nc.vector.tensor_mul(t1t[:], t1t[:], ss[:, sl])
                nc.vector.tensor_scalar(
                    t1t[:], t1t[:], -0.5, 1.5, op0=Alu.mult, op1=Alu.add
                )
                nc.vector.tensor_mul(z[:, sl], z[:, sl], t1t[:])

            def pro_fill(xT, t0, t1_, tag):
                lds, ss = _state[tag]
                z = _z[tag]
                on_act = tag == "0"
                for t in range(t0, t1_):
                    nb = sqpool.tile([128, 128], bf16, tag="nb")
                    if on_act:
                        nc.scalar.activation(
                            nb[:], lds[t][:], Act.Copy, scale=z[:, t : t + 1]
                        )
                    else:
                        nc.vector.tensor_scalar(
                            nb[:], lds[t][:], z[:, t : t + 1], None, op0=Alu.mult
                        )
                    pt = psum.tile([128, 128], bf16, tag="ps")
                    nc.tensor.transpose(pt[:], nb[:], idt[:])
                    if on_act:
                        nc.scalar.copy(xT[:, ts(t, 128)], pt[:])
                    else:
                        nc.vector.tensor_copy(xT[:, ts(t, 128)], pt[:])
                    lds[t] = None

            _state = {
                "0": (
                    [None] * RTN,
                    persist.tile([128, RTN], f32, tag="ss_0", name="ss_0"),
                ),
                "1": (
                    [None] * RTM,
                    persist.tile([128, RTM], f32, tag="ss_1", name="ss_1"),
                ),
            }
            _z = {
                "0": persist.tile([128, RTN], f32, tag="z_0", name="z_0"),
                "1": persist.tile([128, RTM], f32, tag="z_1", name="z_1"),
            }
            HN, HM = (RTN + 1) // 2, (RTM + 1) // 2
            # first halves of both tensors (phase-A dependencies)
            pro_squares(d0, 0, HN, "0")
            pro_squares(d1, 0, HM, "1")
            pro_norm(0, HN, "0")
            pro_norm(0, HM, "1")
            pro_fill(d0T, 0, HN, "0")
            pro_fill(d1T, 0, HM, "1")

            h8_0 = persist.tile([128, RTN * NCH_N * 8], f32, tag="h80")
            h8_1 = persist.tile([128, RTM * NCH_M * 8], f32, tag="h81")

            # ------------- main: matmul halves + evac + max8 ------------------
            def half_tile(lhsT, rhsT, rt, h, nch, h8, sim_dram, sqtag):
                ps = psum.tile([128, half], f32, tag="ps")
                for c in range(half // 512):
                    off = h * half + c * 512
                    nc.tensor.matmul(
                        ps[:, ts(c, 512)],
                        lhsT[:, ts(rt, 128)],
                        rhsT[:, off : off + 512],
                        start=True,
                        stop=True,
                    )
                sq = simpool.tile([128, half], f32, tag=sqtag)
                nc.scalar.copy(sq[:], ps[:])
                nc.vector.max(
                    out=h8[:, (rt * nch + h) * 8 : (rt * nch + h + 1) * 8],
                    in_=sq[:],
                )
                if sim_dram is not None:
                    nc.sync.dma_start(
                        sim_dram.ap()[ts(rt, 128), ts(h, half)], sq[:]
                    )

            def d0_half(rt, h):
                half_tile(d0T, d1T, rt, h, NCH_N, h8_0, sim_o, "sq0")

            def d1_half(rt, h):
                half_tile(d1T, d0T, rt, h, NCH_M, h8_1, None, "sq1")

            # phase A: first-half rowtiles x first-half columns of both
            # directions — depends only on the first-half prologues. The
            # second-half prologue work is interleaved into phase A's emission
            # so ACT/DVE drain it gradually behind the main-loop work.
            phase_a = NCH_N > 1 and NCH_M > 1 and HN == HM and RTN == RTM
            if phase_a:
                nA = HN
                sq_per = -(-HN // (nA // 2))  # squares per iter over first half
                fl_per = -(-HN // (nA - nA // 2))
                for rt in range(nA):
                    d0_half(rt, 0)
                    d1_half(rt, 0)
                    if rt < nA // 2:
                        a = HN + rt * sq_per
                        b = min(RTN, a + sq_per)
                        if a < b:
                            pro_squares(d0, a, b, "0")
                            pro_squares(d1, a, b, "1")
                        if b == RTN:
                            pass
                    if rt == nA // 2 - 1:
                        pro_norm(HN, RTN, "0")
                        pro_norm(HM, RTM, "1")
                    if rt >= nA // 2:
                        a = HN + (rt - nA // 2) * fl_per
                        b = min(RTN, a + fl_per)
                        if a < b:
                            pro_fill(d0T, a, b, "0")
                            pro_fill(d1T, a, b, "1")
            else:
                pro_squares(d0, HN, RTN, "0")
                pro_squares(d1, HM, RTM, "1")
                pro_norm(HN, RTN, "0")
                pro_norm(HM, RTM, "1")
                pro_fill(d0T, HN, RTN, "0")
                pro_fill(d1T, HM, RTM, "1")

            # phase B: everything else
            for rt in range(max(RTN, RTM)):
                if rt < RTN:
                    for h in range(NCH_N):
                        if rt < HN and h == 0 and phase_a:
                            continue
                        d0_half(rt, h)
                if rt < RTM:
                    for h in range(NCH_M):
                        if rt < HM and h == 0 and phase_a:
                            continue
                        d1_half(rt, h)

            nc.sync.dma_start(
                th0.ap().rearrange("a (t p k) -> p a t k", p=128, k=NCH_N * 8),
                h8_0[:].rearrange("p (t k) -> p t k", k=NCH_N * 8),
            )
            nc.sync.dma_start(
                th1.ap().rearrange("a (t p k) -> p a t k", p=128, k=NCH_M * 8),
                h8_1[:].rearrange("p (t k) -> p t k", k=NCH_M * 8),
            )

    nc.compile()
    return nc


def _get_nc(n=N, m=M, d=D):
    key = (n, m, d)
    if key not in _CACHE:
        _CACHE[key] = _build(n, m, d)
    return _CACHE[key]


def _find_nn_host(cand, sim_rows):
    """cand: [R, K] top candidates per row; sim_rows: callable i -> sim row.

    Returns pre-mutual matches [R] int32 (-1 or argmax index)."""
    r2 = RATIO_THRESH * RATIO_THRESH
    d2 = DIST_THRESH * DIST_THRESH
    part = np.partition(cand, cand.shape[1] - 2, axis=1)
    v1 = part[:, -1]
    v2 = part[:, -2]
    dist1 = 2.0 * (1.0 - v1)
    dist2 = 2.0 * (1.0 - v2)
    mask = (dist1 <= r2 * dist2) & (dist1 <= d2)
    out = np.full(cand.shape[0], -1, dtype=np.int32)
    for i in np.nonzero(mask)[0]:
        out[i] = int(np.argmax(sim_rows(int(i))))
    return out


def _mutual_check(m0, m1):
    i0 = np.arange(m0.shape[-1])[None, :]
    i1 = np.arange(m1.shape[-1])[None, :]
    loop0 = np.take_along_axis(m1, np.where(m0 > -1, m0, 0), axis=-1)
    loop1 = np.take_along_axis(m0, np.where(m1 > -1, m1, 0), axis=-1)
    m0n = np.where((m0 > -1) & (i0 == loop0), m0, -1)
    m1n = np.where((m1 > -1) & (i1 == loop1), m1, -1)
    return m0n, m1n


def _postprocess(res, n=N, m=M):
    half = min(HALF, m)
    nch_n, nch_m = m // half, n // half
    sim = np.stack([res.results[b]["sim"] for b in range(B)])
    m0 = np.empty((B, n), dtype=np.int32)
    m1 = np.empty((B, m), dtype=np.int32)
    for b in range(B):
        c0 = res.results[b]["top8h0"].reshape(n // 128, 128, nch_n * 8)
        c0 = c0.reshape(n, nch_n * 8)
        c1 = res.results[b]["top8h1"].reshape(m // 128, 128, nch_m * 8)
        c1 = c1.reshape(m, nch_m * 8)
        m0[b] = _find_nn_host(c0, lambda i: sim[b, i, :])
        m1[b] = _find_nn_host(c1, lambda j: sim[b, :, j])
    m0, m1 = _mutual_check(m0, m1)
    ms0 = (m0 > -1).astype(np.float32)
    ms1 = (m1 > -1).astype(np.float32)
    return (
        m0.astype(np.int32),
        m1.astype(np.int32),
        ms0,
        ms1,
        sim.astype(np.float32),
    )


def _execute(descriptors0, descriptors1, trace=False, trace_cores=None):
    from concourse.bass_utils import run_bass_kernel_spmd

    assert descriptors0.shape == (B, N, D) and descriptors1.shape == (B, M, D)
    nc = _get_nc()
    in_maps = [
        {
            "descriptors0": np.ascontiguousarray(descriptors0[b], dtype=np.float32),
            "descriptors1": np.ascontiguousarray(descriptors1[b], dtype=np.float32),
        }
        for b in range(B)
    ]
    res = run_bass_kernel_spmd(
        nc, in_maps, core_ids=list(range(B)), trace=trace, trace_cores=trace_cores
    )
    return _postprocess(res), res


def kernel(descriptors0: np.ndarray, descriptors1: np.ndarray):
    out, _ = _execute(descriptors0, descriptors1)
    return out


# revision 21
# speedup vs baseline: 1.0014x; 1.0014x over previous
"""Trainium2 Bass kernel for NearestNeighborMatcher (retrieval_knn).

Contract: kernel(**inputs) takes FULL inputs (B=8 batches), shards one batch
element per NeuronCore (8 cores, data-parallel, no collectives), and returns
the FULL output tuple (matches0, matches1, mscores0, mscores1, sim).

Device computes per core: normalized bf16 descriptor transposes, the two
4096x4096 similarity matmuls (sim and simT), fp32 sim written to HBM, and
per-row top-8 candidates of every 2048-wide half via the DVE MAX8 unit for
both directions. Host finishes with O(n) work: top-2 merge, ratio/distance
thresholds, argmax lookup for the (rare) mask-passing rows from the sim
output itself, and the mutual check.
"""

import sys

sys.path.insert(0, "/opt/trn_rl_repo")

import numpy as np

B, N, M, D = 8, 4096, 4096, 128
RATIO_THRESH = 0.8
DIST_THRESH = 0.7
EPS = 1e-12
HALF = 2048  # PSUM accumulation / evacuation granularity

_CACHE = {}


def _build(n=N, m=M, d=D):
    import concourse.bass as bass
    import concourse.mybir as mybir
    from concourse import bacc, tile
    from concourse.bass import ts
    from concourse.masks import make_identity

    f32 = mybir.dt.float32
    bf16 = mybir.dt.bfloat16
    Alu = mybir.AluOpType
    Act = mybir.ActivationFunctionType

    assert d == 128 and n % 128 == 0 and m % 128 == 0
    RTN, RTM = n // 128, m // 128
    half = min(HALF, m)
    assert n % half == 0 and m % half == 0
    NCH_N, NCH_M = m // half, n // half  # halves per row, per direction

    nc = bacc.Bacc(
        "TRN2", target_bir_lowering=False, debug=False, enable_asserts=True
    )
    d0 = nc.dram_tensor("descriptors0", [n, d], f32, kind="ExternalInput")
    d1 = nc.dram_tensor("descriptors1", [m, d], f32, kind="ExternalInput")
    sim_o = nc.dram_tensor("sim", [n, m], f32, kind="ExternalOutput")
    th0 = nc.dram_tensor("top8h0", [1, n * NCH_N * 8], f32, kind="ExternalOutput")
    th1 = nc.dram_tensor("top8h1", [1, m * NCH_M * 8], f32, kind="ExternalOutput")

    with tile.TileContext(nc) as tc:
        import contextlib

        ctx = contextlib.ExitStack()
        with ctx:
            persist = ctx.enter_context(tc.tile_pool(name="persist", bufs=1))
            ldpool = ctx.enter_context(tc.tile_pool(name="ld", bufs=RTN + RTM))
            sqpool = ctx.enter_context(tc.tile_pool(name="sq", bufs=2))
            psum = ctx.enter_context(
                tc.tile_pool(name="psum", bufs=2, space="PSUM")
            )
            simpool = ctx.enter_context(tc.tile_pool(name="simbuf", bufs=4))

            idt = persist.tile([128, 128], bf16, tag="idt")
            make_identity(nc, idt[:])

            d0T = persist.tile([128, n], bf16, tag="d0T")
            d1T = persist.tile([128, m], bf16, tag="d1T")

            # ---------------- prologue: normalize + transpose ----------------
            # Split into per-half-tensor pieces so the main loop's first
            # quadrant can start after only the first halves are built.
            def pro_squares(src, t0, t1_, tag):
                lds, ss = _state[tag]
                on_act = tag == "0"
                for t in range(t0, t1_):
                    ld = lds[t]
                    sq = sqpool.tile([128, 128], f32, tag="sq")
                    if on_act:
                        nc.scalar.activation(
                            sq[:], ld[:], Act.Square, accum_out=ss[:, t : t + 1]
                        )
                    else:
                        nc.vector.scalar_tensor_tensor(
                            sq[:],
                            ld[:],
                            0.0,
                            ld[:],
                            op0=Alu.add,
                            op1=Alu.mult,
                            accum_out=ss[:, t : t + 1],
                        )

            def pro_norm(t0, t1_, tag):
                # z = 1/max(sqrt(ss), eps) with one Newton step on rsqrt
                lds, ss = _state[tag]
                w = t1_ - t0
                sl = slice(t0, t1_)
                sroot = sqpool.tile([128, w], f32, tag="sr")
                nc.scalar.activation(sroot[:], ss[:, sl], Act.Sqrt)
                nc.vector.tensor_scalar_max(sroot[:], sroot[:], float(EPS))
                z = _z[tag]
                nc.vector.reciprocal(z[:, sl], sroot[:])
                t1t = sqpool.tile([128, w], f32, tag="nt1")
                nc.vector.tensor_mul(t1t[:], z[:, sl], z[:, sl])
                nc.vector.tensor_mul(t1t[:], t1t[:], ss[:, sl])
                nc.vector.tensor_scalar(
                    t1t[:], t1t[:], -0.5, 1.5, op0=Alu.mult, op1=Alu.add
                )
                nc.vector.tensor_mul(z[:, sl], z[:, sl], t1t[:])

            def pro_fill(xT, t0, t1_, tag):
                lds, ss = _state[tag]
                z = _z[tag]
                on_act = tag == "0"
                for t in range(t0, t1_):
                    nb = sqpool.tile([128, 128], bf16, tag="nb")
                    if on_act:
                        nc.scalar.activation(
                            nb[:], lds[t][:], Act.Copy, scale=z[:, t : t + 1]
                        )
                    else:
                        nc.vector.tensor_scalar(
                            nb[:], lds[t][:], z[:, t : t + 1], None, op0=Alu.mult
                        )
                    pt = psum.tile([128, 128], bf16, tag="ps")
                    nc.tensor.transpose(pt[:], nb[:], idt[:])
                    if on_act:
                        nc.scalar.copy(xT[:, ts(t, 128)], pt[:])
                    else:
                        nc.vector.tensor_copy(xT[:, ts(t, 128)], pt[:])
                    lds[t] = None

            _state = {
                "0": (
                    [None] * RTN,
                    persist.tile([128, RTN], f32, tag="ss_0", name="ss_0"),
                ),
                "1": (
                    [None] * RTM,
                    persist.tile([128, RTM], f32, tag="ss_1", name="ss_1"),
                ),
            }
            _z = {
                "0": persist.tile([128, RTN], f32, tag="z_0", name="z_0"),
                "1": persist.tile([128, RTM], f32, tag="z_1", name="z_1"),
            }
            # prefetch every descriptor tile upfront, interleaving tensors and
            # alternating DMA queues so both streams arrive early
            for t in range(max(RTN, RTM)):
                for tag, src, cnt in (("0", d0, RTN), ("1", d1, RTM)):
                    if t < cnt:
                        ld = ldpool.tile(
                            [128, 128], f32, tag="ld", name=f"ld{tag}_{t}"
                        )
                        eng = nc.sync if (t % 2 == 0) == (tag == "0") else nc.gpsimd
                        eng.dma_start(ld[:], src.ap()[ts(t, 128), :])
                        _state[tag][0][t] = ld
            HN, HM = (RTN + 1) // 2, (RTM + 1) // 2
            # first halves of both tensors (phase-A dependencies)
            pro_squares(d0, 0, HN, "0")
            pro_squares(d1, 0, HM, "1")
            pro_norm(0, HN, "0")
            pro_norm(0, HM, "1")
            pro_fill(d0T, 0, HN, "0")
            pro_fill(d1T, 0, HM, "1")

            h8_0 = persist.tile([128, RTN * NCH_N * 8], f32, tag="h80")
            h8_1 = persist.tile([128, RTM * NCH_M * 8], f32, tag="h81")

            # ------------- main: matmul halves + evac + max8 ------------------
            def half_tile(lhsT, rhsT, rt, h, nch, h8, sim_dram, sqtag):
                ps = psum.tile([128, half], f32, tag="ps")
                for c in range(half // 512):
                    off = h * half + c * 512
                    nc.tensor.matmul(
                        ps[:, ts(c, 512)],
                        lhsT[:, ts(rt, 128)],
                        rhsT[:, off : off + 512],
                        start=True,
                        stop=True,
                    )
                sq = simpool.tile([128, half], f32, tag=sqtag)
                nc.scalar.copy(sq[:], ps[:])
                nc.vector.max(
                    out=h8[:, (rt * nch + h) * 8 : (rt * nch + h + 1) * 8],
                    in_=sq[:],
                )
                if sim_dram is not None:
                    nc.sync.dma_start(
                        sim_dram.ap()[ts(rt, 128), ts(h, half)], sq[:]
                    )

            def d0_half(rt, h):
                half_tile(d0T, d1T, rt, h, NCH_N, h8_0, sim_o, "sq0")

            def d1_half(rt, h):
                half_tile(d1T, d0T, rt, h, NCH_M, h8_1, None, "sq1")

            # phase A: first-half rowtiles x first-half columns of both
            # directions — depends only on the first-half prologues. The
            # second-half prologue work is interleaved into phase A's emission
            # so ACT/DVE drain it gradually behind the main-loop work.
            phase_a = NCH_N > 1 and NCH_M > 1 and HN == HM and RTN == RTM
            if phase_a:
                nA = HN
                sq_per = -(-HN // (nA // 2))  # squares per iter over first half
                fl_per = -(-HN // (nA - nA // 2))
                for rt in range(nA):
                    d0_half(rt, 0)
                    d1_half(rt, 0)
                    if rt < nA // 2:
                        a = HN + rt * sq_per
                        b = min(RTN, a + sq_per)
                        if a < b:
                            pro_squares(d0, a, b, "0")
                            pro_squares(d1, a, b, "1")
                        if b == RTN:
                            pass
                    if rt == nA // 2 - 1:
                        pro_norm(HN, RTN, "0")
                        pro_norm(HM, RTM, "1")
                    if rt >= nA // 2:
                        a = HN + (rt - nA // 2) * fl_per
                        b = min(RTN, a + fl_per)
                        if a < b:
                            pro_fill(d0T, a, b, "0")
                            pro_fill(d1T, a, b, "1")
            else:
                pro_squares(d0, HN, RTN, "0")
                pro_squares(d1, HM, RTM, "1")
                pro_norm(HN, RTN, "0")
                pro_norm(HM, RTM, "1")
                pro_fill(d0T, HN, RTN, "0")
                pro_fill(d1T, HM, RTM, "1")

            # phase B: everything else
            for rt in range(max(RTN, RTM)):
                if rt < RTN:
                    for h in range(NCH_N):
                        if rt < HN and h == 0 and phase_a:
                            continue
                        d0_half(rt, h)
                if rt < RTM:
                    for h in range(NCH_M):
                        if rt < HM and h == 0 and phase_a:
                            continue
                        d1_half(rt, h)

            nc.sync.dma_start(
                th0.ap().rearrange("a (t p k) -> p a t k", p=128, k=NCH_N * 8),
                h8_0[:].rearrange("p (t k) -> p t k", k=NCH_N * 8),
            )
            nc.sync.dma_start(
                th1.ap().rearrange("a (t p k) -> p a t k", p=128, k=NCH_M * 8),
                h8_1[:].rearrange("p (t k) -> p t k", k=NCH_M * 8),
            )

    nc.compile()
    return nc


def _get_nc(n=N, m=M, d=D):
    key = (n, m, d)
    if key not in _CACHE:
        _CACHE[key] = _build(n, m, d)
    return _CACHE[key]


def _find_nn_host(cand, sim_rows):
    """cand: [R, K] top candidates per row; sim_rows: callable i -> sim row.

    Returns pre-mutual matches [R] int32 (-1 or argmax index)."""
    r2 = RATIO_THRESH * RATIO_THRESH
    d2 = DIST_THRESH * DIST_THRESH
    part = np.partition(cand, cand.shape[1] - 2, axis=1)
    v1 = part[:, -1]
    v2 = part[:, -2]
    dist1 = 2.0 * (1.0 - v1)
    dist2 = 2.0 * (1.0 - v2)
    mask = (dist1 <= r2 * dist2) & (dist1 <= d2)
    out = np.full(cand.shape[0], -1, dtype=np.int32)
    for i in np.nonzero(mask)[0]:
        out[i] = int(np.argmax(sim_rows(int(i))))
    return out


def _mutual_check(m0, m1):
    i0 = np.arange(m0.shape[-1])[None, :]
    i1 = np.arange(m1.shape[-1])[None, :]
    loop0 = np.take_along_axis(m1, np.where(m0 > -1, m0, 0), axis=-1)
    loop1 = np.take_along_axis(m0, np.where(m1 > -1, m1, 0), axis=-1)
    m0n = np.where((m0 > -1) & (i0 == loop0), m0, -1)
    m1n = np.where((m1 > -1) & (i1 == loop1), m1, -1)
    return m0n, m1n


def _postprocess(res, n=N, m=M):
    half = min(HALF, m)
    nch_n, nch_m = m // half, n // half
    sim = np.stack([res.results[b]["sim"] for b in range(B)])
    m0 = np.empty((B, n), dtype=np.int32)
    m1 = np.empty((B, m), dtype=np.int32)
    for b in range(B):
        c0 = res.results[b]["top8h0"].reshape(n // 128, 128, nch_n * 8)
        c0 = c0.reshape(n, nch_n * 8)
        c1 = res.results[b]["top8h1"].reshape(m // 128, 128, nch_m * 8)
        c1 = c1.reshape(m, nch_m * 8)
        m0[b] = _find_nn_host(c0, lambda i: sim[b, i, :])
        m1[b] = _find_nn_host(c1, lambda j: sim[b, :, j])
    m0, m1 = _mutual_check(m0, m1)
    ms0 = (m0 > -1).astype(np.float32)
    ms1 = (m1 > -1).astype(np.float32)
    return (
        m0.astype(np.int32),
        m1.astype(np.int32),
        ms0,
        ms1,
        sim.astype(np.float32),
    )


def _execute(descriptors0, descriptors1, trace=False, trace_cores=None):
    from concourse.bass_utils import run_bass_kernel_spmd

    assert descriptors0.shape == (B, N, D) and descriptors1.shape == (B, M, D)
    nc = _get_nc()
    in_maps = [
        {
            "descriptors0": np.ascontiguousarray(descriptors0[b], dtype=np.float32),
            "descriptors1": np.ascontiguousarray(descriptors1[b], dtype=np.float32),
        }
        for b in range(B)
    ]
    res = run_bass_kernel_spmd(
        nc, in_maps, core_ids=list(range(B)), trace=trace, trace_cores=trace_cores
    )
    return _postprocess(res), res


def kernel(descriptors0: np.ndarray, descriptors1: np.ndarray):
    out, _ = _execute(descriptors0, descriptors1)
    return out


# revision 22
# speedup vs baseline: 1.0756x; 1.0741x over previous
"""Trainium2 Bass kernel for NearestNeighborMatcher (retrieval_knn).

Contract: kernel(**inputs) takes FULL inputs (B=8 batches), shards one batch
element per NeuronCore (8 cores, data-parallel, no collectives), and returns
the FULL output tuple (matches0, matches1, mscores0, mscores1, sim).

Device computes per core: normalized bf16 descriptor transposes, the two
4096x4096 similarity matmuls (sim and simT), fp32 sim written to HBM, and
per-row top-8 candidates of every 2048-wide half via the DVE MAX8 unit for
both directions. Host finishes with O(n) work: top-2 merge, ratio/distance
thresholds, argmax lookup for the (rare) mask-passing rows from the sim
output itself, and the mutual check.
"""

import sys

sys.path.insert(0, "/opt/trn_rl_repo")

import numpy as np

B, N, M, D = 8, 4096, 4096, 128
RATIO_THRESH = 0.8
DIST_THRESH = 0.7
EPS = 1e-12
HALF = 2048  # PSUM accumulation / evacuation granularity

_CACHE = {}


def _build(n=N, m=M, d=D):
    import concourse.bass as bass
    import concourse.mybir as mybir
    from concourse import bacc, tile
    from concourse.bass import ts
    from concourse.masks import make_identity

    f32 = mybir.dt.float32
    bf16 = mybir.dt.bfloat16
    Alu = mybir.AluOpType
    Act = mybir.ActivationFunctionType

    assert d == 128 and n % 128 == 0 and m % 128 == 0
    RTN, RTM = n // 128, m // 128
    half = min(HALF, m)
    assert n % half == 0 and m % half == 0
    NCH_N, NCH_M = m // half, n // half  # halves per row, per direction

    nc = bacc.Bacc(
        "TRN2", target_bir_lowering=False, debug=False, enable_asserts=True
    )
    d0 = nc.dram_tensor("descriptors0", [n, d], f32, kind="ExternalInput")
    d1 = nc.dram_tensor("descriptors1", [m, d], f32, kind="ExternalInput")
    sim_o = nc.dram_tensor("sim", [n, m], f32, kind="ExternalOutput")
    th0 = nc.dram_tensor("top8h0", [1, n * NCH_N * 8], f32, kind="ExternalOutput")
    th1 = nc.dram_tensor("top8h1", [1, m * NCH_M * 8], f32, kind="ExternalOutput")

    with tile.TileContext(nc) as tc:
        import contextlib

        ctx = contextlib.ExitStack()
        with ctx:
            persist = ctx.enter_context(tc.tile_pool(name="persist", bufs=1))
            ldpool = ctx.enter_context(tc.tile_pool(name="ld", bufs=RTN + RTM))
            sqpool = ctx.enter_context(tc.tile_pool(name="sq", bufs=2))
            psum = ctx.enter_context(
                tc.tile_pool(name="psum", bufs=2, space="PSUM")
            )
            simpool = ctx.enter_context(tc.tile_pool(name="simbuf", bufs=4))

            idt = persist.tile([128, 128], bf16, tag="idt")
            make_identity(nc, idt[:])

            d0T = persist.tile([128, n], bf16, tag="d0T")
            d1T = persist.tile([128, m], bf16, tag="d1T")

            # ---------------- prologue: normalize + transpose ----------------
            # Split into per-half-tensor pieces so the main loop's first
            # quadrant can start after only the first halves are built.
            def pro_squares(src, t0, t1_, tag):
                lds, ss = _state[tag]
                on_act = tag == "0"
                for t in range(t0, t1_):
                    ld = lds[t]
                    sq = sqpool.tile([128, 128], f32, tag="sq")
                    if on_act:
                        nc.scalar.activation(
                            sq[:], ld[:], Act.Square, accum_out=ss[:, t : t + 1]
                        )
                    else:
                        nc.vector.scalar_tensor_tensor(
                            sq[:],
                            ld[:],
                            0.0,
                            ld[:],
                            op0=Alu.add,
                            op1=Alu.mult,
                            accum_out=ss[:, t : t + 1],
                        )

            def pro_norm(t0, t1_, tag):
                # z = 1/max(sqrt(ss), eps) with one Newton step on rsqrt
                lds, ss = _state[tag]
                w = t1_ - t0
                sl = slice(t0, t1_)
                sroot = sqpool.tile([128, w], f32, tag="sr")
                nc.scalar.activation(sroot[:], ss[:, sl], Act.Sqrt)
                nc.vector.tensor_scalar_max(sroot[:], sroot[:], float(EPS))
                z = _z[tag]
                nc.vector.reciprocal(z[:, sl], sroot[:])
                t1t = sqpool.tile([128, w], f32, tag="nt1")
                nc.vector.tensor_mul(t1t[:], z[:, sl], z[:, sl])
                nc.vector.tensor_mul(t1t[:], t1t[:], ss[:, sl])
                nc.vector.tensor_scalar(
                    t1t[:], t1t[:], -0.5, 1.5, op0=Alu.mult, op1=Alu.add
                )
                nc.vector.tensor_mul(z[:, sl], z[:, sl], t1t[:])

            def pro_fill(xT, t0, t1_, tag):
                # batched: 4 normalize-muls + 4 PE transposes into one PSUM
                # allocation, one grouped copy out — minimizes PSUM slot churn
                lds, ss = _state[tag]
                z = _z[tag]
                on_act = tag == "0"
                for g in range(t0, t1_, 4):
                    ge = min(g + 4, t1_)
                    w = (ge - g) * 128
                    nb = sqpool.tile(
                        [128, 512], bf16, tag="nb", name=f"nb{tag}_{g}"
                    )
                    for t in range(g, ge):
                        sl = nb[:, ts(t - g, 128)]
                        if on_act:
                            nc.scalar.activation(
                                sl, lds[t][:], Act.Copy, scale=z[:, t : t + 1]
                            )
                        else:
                            nc.vector.tensor_scalar(
                                sl, lds[t][:], z[:, t : t + 1], None, op0=Alu.mult
                            )
                    pt = psum.tile([128, 512], bf16, tag="ps", name=f"pt{tag}_{g}")
                    for t in range(g, ge):
                        nc.tensor.transpose(
                            pt[:, ts(t - g, 128)], nb[:, ts(t - g, 128)], idt[:]
                        )
                        lds[t] = None
                    if on_act:
                        nc.scalar.copy(xT[:, g * 128 : ge * 128], pt[:, :w])
                    else:
                        nc.vector.tensor_copy(xT[:, g * 128 : ge * 128], pt[:, :w])

            _state = {
                "0": (
                    [None] * RTN,
                    persist.tile([128, RTN], f32, tag="ss_0", name="ss_0"),
                ),
                "1": (
                    [None] * RTM,
                    persist.tile([128, RTM], f32, tag="ss_1", name="ss_1"),
                ),
            }
            _z = {
                "0": persist.tile([128, RTN], f32, tag="z_0", name="z_0"),
                "1": persist.tile([128, RTM], f32, tag="z_1", name="z_1"),
            }
            # prefetch every descriptor tile upfront, interleaving tensors and
            # alternating DMA queues so both streams arrive early
            for t in range(max(RTN, RTM)):
                for tag, src, cnt in (("0", d0, RTN), ("1", d1, RTM)):
                    if t < cnt:
                        ld = ldpool.tile(
                            [128, 128], f32, tag="ld", name=f"ld{tag}_{t}"
                        )
                        eng = nc.sync if (t % 2 == 0) == (tag == "0") else nc.gpsimd
                        eng.dma_start(ld[:], src.ap()[ts(t, 128), :])
                        _state[tag][0][t] = ld
            HN, HM = (RTN + 1) // 2, (RTM + 1) // 2
            # first halves of both tensors (phase-A dependencies)
            pro_squares(d0, 0, HN, "0")
            pro_squares(d1, 0, HM, "1")
            pro_norm(0, HN, "0")
            pro_norm(0, HM, "1")
            pro_fill(d0T, 0, HN, "0")
            pro_fill(d1T, 0, HM, "1")

            h8_0 = persist.tile([128, RTN * NCH_N * 8], f32, tag="h80")
            h8_1 = persist.tile([128, RTM * NCH_M * 8], f32, tag="h81")

            # ------------- main: matmul halves + evac + max8 ------------------
            def half_tile(lhsT, rhsT, rt, h, nch, h8, sim_dram, sqtag):
                ps = psum.tile([128, half], f32, tag="ps")
                for c in range(half // 512):
                    off = h * half + c * 512
                    nc.tensor.matmul(
                        ps[:, ts(c, 512)],
                        lhsT[:, ts(rt, 128)],
                        rhsT[:, off : off + 512],
                        start=True,
                        stop=True,
                    )
                sq = simpool.tile([128, half], f32, tag=sqtag)
                nc.scalar.copy(sq[:], ps[:])
                nc.vector.max(
                    out=h8[:, (rt * nch + h) * 8 : (rt * nch + h + 1) * 8],
                    in_=sq[:],
                )
                if sim_dram is not None:
                    nc.sync.dma_start(
                        sim_dram.ap()[ts(rt, 128), ts(h, half)], sq[:]
                    )

            def d0_half(rt, h):
                half_tile(d0T, d1T, rt, h, NCH_N, h8_0, sim_o, "sq0")

            def d1_half(rt, h):
                half_tile(d1T, d0T, rt, h, NCH_M, h8_1, None, "sq1")

            # phase A: first-half rowtiles x first-half columns of both
            # directions — depends only on the first-half prologues. The
            # second-half prologue work is interleaved into phase A's emission
            # so ACT/DVE drain it gradually behind the main-loop work.
            phase_a = NCH_N > 1 and NCH_M > 1 and HN == HM and RTN == RTM
            if phase_a:
                nA = HN
                sq_per = -(-HN // (nA // 2))  # squares per iter over first half
                fl_per = -(-HN // (nA - nA // 2))
                for rt in range(nA):
                    d0_half(rt, 0)
                    d1_half(rt, 0)
                    if rt < nA // 2:
                        a = HN + rt * sq_per
                        b = min(RTN, a + sq_per)
                        if a < b:
                            pro_squares(d0, a, b, "0")
                            pro_squares(d1, a, b, "1")
                        if b == RTN:
                            pass
                    if rt == nA // 2 - 1:
                        pro_norm(HN, RTN, "0")
                        pro_norm(HM, RTM, "1")
                    if rt >= nA // 2:
                        a = HN + (rt - nA // 2) * fl_per
                        b = min(RTN, a + fl_per)
                        if a < b:
                            pro_fill(d0T, a, b, "0")
                            pro_fill(d1T, a, b, "1")
            else:
                pro_squares(d0, HN, RTN, "0")
                pro_squares(d1, HM, RTM, "1")
                pro_norm(HN, RTN, "0")
                pro_norm(HM, RTM, "1")
                pro_fill(d0T, HN, RTN, "0")
                pro_fill(d1T, HM, RTM, "1")

            # phase B: everything else
            for rt in range(max(RTN, RTM)):
                if rt < RTN:
                    for h in range(NCH_N):
                        if rt < HN and h == 0 and phase_a:
                            continue
                        d0_half(rt, h)
                if rt < RTM:
                    for h in range(NCH_M):
                        if rt < HM and h == 0 and phase_a:
                            continue
                        d1_half(rt, h)

            nc.sync.dma_start(
                th0.ap().rearrange("a (t p k) -> p a t k", p=128, k=NCH_N * 8),
                h8_0[:].rearrange("p (t k) -> p t k", k=NCH_N * 8),
            )
            nc.sync.dma_start(
                th1.ap().rearrange("a (t p k) -> p a t k", p=128, k=NCH_M * 8),
                h8_1[:].rearrange("p (t k) -> p t k", k=NCH_M * 8),
            )

    nc.compile()
    return nc


def _get_nc(n=N, m=M, d=D):
    key = (n, m, d)
    if key not in _CACHE:
        _CACHE[key] = _build(n, m, d)
    return _CACHE[key]


def _find_nn_host(cand, sim_rows):
    """cand: [R, K] top candidates per row; sim_rows: callable i -> sim row.

    Returns pre-mutual matches [R] int32 (-1 or argmax index)."""
    r2 = RATIO_THRESH * RATIO_THRESH
    d2 = DIST_THRESH * DIST_THRESH
    part = np.partition(cand, cand.shape[1] - 2, axis=1)
    v1 = part[:, -1]
    v2 = part[:, -2]
    dist1 = 2.0 * (1.0 - v1)
    dist2 = 2.0 * (1.0 - v2)
    mask = (dist1 <= r2 * dist2) & (dist1 <= d2)
    out = np.full(cand.shape[0], -1, dtype=np.int32)
    for i in np.nonzero(mask)[0]:
        out[i] = int(np.argmax(sim_rows(int(i))))
    return out


def _mutual_check(m0, m1):
    i0 = np.arange(m0.shape[-1])[None, :]
    i1 = np.arange(m1.shape[-1])[None, :]
    loop0 = np.take_along_axis(m1, np.where(m0 > -1, m0, 0), axis=-1)
    loop1 = np.take_along_axis(m0, np.where(m1 > -1, m1, 0), axis=-1)
    m0n = np.where((m0 > -1) & (i0 == loop0), m0, -1)
    m1n = np.where((m1 > -1) & (i1 == loop1), m1, -1)
    return m0n, m1n


def _postprocess(res, n=N, m=M):
    half = min(HALF, m)
    nch_n, nch_m = m // half, n // half
    sim = np.stack([res.results[b]["sim"] for b in range(B)])
    m0 = np.empty((B, n), dtype=np.int32)
    m1 = np.empty((B, m), dtype=np.int32)
    for b in range(B):
        c0 = res.results[b]["top8h0"].reshape(n // 128, 128, nch_n * 8)
        c0 = c0.reshape(n, nch_n * 8)
        c1 = res.results[b]["top8h1"].reshape(m // 128, 128, nch_m * 8)
        c1 = c1.reshape(m, nch_m * 8)
        m0[b] = _find_nn_host(c0, lambda i: sim[b, i, :])
        m1[b] = _find_nn_host(c1, lambda j: sim[b, :, j])
    m0, m1 = _mutual_check(m0, m1)
    ms0 = (m0 > -1).astype(np.float32)
    ms1 = (m1 > -1).astype(np.float32)
    return (
        m0.astype(np.int32),
        m1.astype(np.int32),
        ms0,
        ms1,
        sim.astype(np.float32),
    )


def _execute(descriptors0, descriptors1, trace=False, trace_cores=None):
    from concourse.bass_utils import run_bass_kernel_spmd

    assert descriptors0.shape == (B, N, D) and descriptors1.shape == (B, M, D)
    nc = _get_nc()
    in_maps = [
        {
            "descriptors0": np.ascontiguousarray(descriptors0[b], dtype=np.float32),
            "descriptors1": np.ascontiguousarray(descriptors1[b], dtype=np.float32),
        }
        for b in range(B)
    ]
    res = run_bass_kernel_spmd(
        nc, in_maps, core_ids=list(range(B)), trace=trace, trace_cores=trace_cores
    )
    return _postprocess(res), res


def kernel(descriptors0: np.ndarray, descriptors1: np.ndarray):
    out, _ = _execute(descriptors0, descriptors1)
    return out


# revision 25
# speedup vs baseline: 1.1108x; 1.0327x over previous
"""Trainium2 Bass kernel for NearestNeighborMatcher (retrieval_knn).

Contract: kernel(**inputs) takes FULL inputs (B=8 batches), shards one batch
element per NeuronCore (8 cores, data-parallel, no collectives), and returns
the FULL output tuple (matches0, matches1, mscores0, mscores1, sim).

Device computes per core: normalized bf16 descriptor transposes, the two
4096x4096 similarity matmuls (sim and simT), fp32 sim written to HBM, and
per-row top-8 candidates of every 2048-wide half via the DVE MAX8 unit for
both directions. Host finishes with O(n) work: top-2 merge, ratio/distance
thresholds, argmax lookup for the (rare) mask-passing rows from the sim
output itself, and the mutual check.
"""

import sys

sys.path.insert(0, "/opt/trn_rl_repo")

import numpy as np

B, N, M, D = 8, 4096, 4096, 128
RATIO_THRESH = 0.8
DIST_THRESH = 0.7
EPS = 1e-12
HALF = 2048  # PSUM accumulation / evacuation granularity

_CACHE = {}


def _build(n=N, m=M, d=D):
    import concourse.bass as bass
    import concourse.mybir as mybir
    from concourse import bacc, tile
    from concourse.bass import ts
    from concourse.masks import make_identity

    f32 = mybir.dt.float32
    bf16 = mybir.dt.bfloat16
    Alu = mybir.AluOpType
    Act = mybir.ActivationFunctionType

    assert d == 128 and n % 128 == 0 and m % 128 == 0
    RTN, RTM = n // 128, m // 128
    half = min(HALF, m)
    assert n % half == 0 and m % half == 0
    NCH_N, NCH_M = m // half, n // half  # halves per row, per direction

    nc = bacc.Bacc(
        "TRN2", target_bir_lowering=False, debug=False, enable_asserts=True
    )
    d0 = nc.dram_tensor("descriptors0", [n, d], f32, kind="ExternalInput")
    d1 = nc.dram_tensor("descriptors1", [m, d], f32, kind="ExternalInput")
    sim_o = nc.dram_tensor("sim", [n, m], f32, kind="ExternalOutput")
    th0 = nc.dram_tensor("top8h0", [1, n * NCH_N * 8], f32, kind="ExternalOutput")
    th1 = nc.dram_tensor("top8h1", [1, m * NCH_M * 8], f32, kind="ExternalOutput")

    with tile.TileContext(nc) as tc:
        import contextlib

        ctx = contextlib.ExitStack()
        with ctx:
            persist = ctx.enter_context(tc.tile_pool(name="persist", bufs=1))
            ldpool = ctx.enter_context(tc.tile_pool(name="ld", bufs=RTN + RTM))
            sqpool = ctx.enter_context(tc.tile_pool(name="sq", bufs=2))
            psum = ctx.enter_context(
                tc.tile_pool(name="psum", bufs=2, space="PSUM")
            )
            simpool = ctx.enter_context(tc.tile_pool(name="simbuf", bufs=4))

            idt = persist.tile([128, 128], bf16, tag="idt")
            make_identity(nc, idt[:])

            d0T = persist.tile([128, n], bf16, tag="d0T")
            d1T = persist.tile([128, m], bf16, tag="d1T")

            # ---------------- prologue: normalize + transpose ----------------
            # Split into per-half-tensor pieces so the main loop's first
            # quadrant can start after only the first halves are built.
            def pro_squares(src, t0, t1_, tag, force_dve=False):
                lds, ss = _state[tag]
                on_act = tag == "0" and not force_dve
                for t in range(t0, t1_):
                    ld = lds[t]
                    sq = sqpool.tile([128, 128], f32, tag="sq")
                    if on_act:
                        nc.scalar.activation(
                            sq[:], ld[:], Act.Square, accum_out=ss[:, t : t + 1]
                        )
                    else:
                        nc.vector.scalar_tensor_tensor(
                            sq[:],
                            ld[:],
                            0.0,
                            ld[:],
                            op0=Alu.add,
                            op1=Alu.mult,
                            accum_out=ss[:, t : t + 1],
                        )

            def pro_norm(t0, t1_, tag):
                # z = 1/max(sqrt(ss), eps) with one Newton step on rsqrt
                lds, ss = _state[tag]
                w = t1_ - t0
                sl = slice(t0, t1_)
                sroot = sqpool.tile([128, w], f32, tag="sr")
                nc.scalar.activation(sroot[:], ss[:, sl], Act.Sqrt)
                nc.vector.tensor_scalar_max(sroot[:], sroot[:], float(EPS))
                z = _z[tag]
                nc.vector.reciprocal(z[:, sl], sroot[:])
                t1t = sqpool.tile([128, w], f32, tag="nt1")
                nc.vector.tensor_mul(t1t[:], z[:, sl], z[:, sl])
                nc.vector.tensor_mul(t1t[:], t1t[:], ss[:, sl])
                nc.vector.tensor_scalar(
                    t1t[:], t1t[:], -0.5, 1.5, op0=Alu.mult, op1=Alu.add
                )
                nc.vector.tensor_mul(z[:, sl], z[:, sl], t1t[:])

            def pro_fill(xT, t0, t1_, tag):
                # batched: 4 normalize-muls + 4 PE transposes into one PSUM
                # allocation, one grouped copy out — minimizes PSUM slot churn
                lds, ss = _state[tag]
                z = _z[tag]
                on_act = tag == "0"
                for g in range(t0, t1_, 4):
                    ge = min(g + 4, t1_)
                    w = (ge - g) * 128
                    nb = sqpool.tile(
                        [128, 512], bf16, tag="nb", name=f"nb{tag}_{g}"
                    )
                    for t in range(g, ge):
                        sl = nb[:, ts(t - g, 128)]
                        if on_act:
                            nc.scalar.activation(
                                sl, lds[t][:], Act.Copy, scale=z[:, t : t + 1]
                            )
                        else:
                            nc.vector.tensor_scalar(
                                sl, lds[t][:], z[:, t : t + 1], None, op0=Alu.mult
                            )
                    pt = psum.tile([128, 512], bf16, tag="ps", name=f"pt{tag}_{g}")
                    for t in range(g, ge):
                        nc.tensor.transpose(
                            pt[:, ts(t - g, 128)], nb[:, ts(t - g, 128)], idt[:]
                        )
                        lds[t] = None
                    if on_act:
                        nc.scalar.copy(xT[:, g * 128 : ge * 128], pt[:, :w])
                    else:
                        nc.vector.tensor_copy(xT[:, g * 128 : ge * 128], pt[:, :w])

            _state = {
                "0": (
                    [None] * RTN,
                    persist.tile([128, RTN], f32, tag="ss_0", name="ss_0"),
                ),
                "1": (
                    [None] * RTM,
                    persist.tile([128, RTM], f32, tag="ss_1", name="ss_1"),
                ),
            }
            _z = {
                "0": persist.tile([128, RTN], f32, tag="z_0", name="z_0"),
                "1": persist.tile([128, RTM], f32, tag="z_1", name="z_1"),
            }
            # prefetch every descriptor tile upfront, interleaving tensors and
            # alternating DMA queues so both streams arrive early
            for t in range(max(RTN, RTM)):
                for tag, src, cnt in (("0", d0, RTN), ("1", d1, RTM)):
                    if t < cnt:
                        ld = ldpool.tile(
                            [128, 128], f32, tag="ld", name=f"ld{tag}_{t}"
                        )
                        eng = nc.sync if (t % 2 == 0) == (tag == "0") else nc.gpsimd
                        eng.dma_start(ld[:], src.ap()[ts(t, 128), :])
                        _state[tag][0][t] = ld
            HN, HM = (RTN + 1) // 2, (RTM + 1) // 2
            # first halves of both tensors (phase-A dependencies)
            pro_squares(d0, 0, HN, "0")
            pro_squares(d1, 0, HM, "1")
            pro_norm(0, HN, "0")
            pro_norm(0, HM, "1")
            pro_fill(d0T, 0, HN, "0")
            pro_fill(d1T, 0, HM, "1")

            h8_0 = persist.tile([128, RTN * NCH_N * 8], f32, tag="h80")
            h8_1 = persist.tile([128, RTM * NCH_M * 8], f32, tag="h81")

            # ------------- main: matmul halves + evac + max8 ------------------
            def half_tile(lhsT, rhsT, rt, h, nch, h8, sim_dram, sqtag):
                ps = psum.tile([128, half], f32, tag="ps")
                for c in range(half // 512):
                    off = h * half + c * 512
                    nc.tensor.matmul(
                        ps[:, ts(c, 512)],
                        lhsT[:, ts(rt, 128)],
                        rhsT[:, off : off + 512],
                        start=True,
                        stop=True,
                    )
                sq = simpool.tile([128, half], f32, tag=sqtag)
                nc.scalar.copy(sq[:], ps[:])
                nc.vector.max(
                    out=h8[:, (rt * nch + h) * 8 : (rt * nch + h + 1) * 8],
                    in_=sq[:],
                )
                if sim_dram is not None:
                    nc.sync.dma_start(
                        sim_dram.ap()[ts(rt, 128), ts(h, half)], sq[:]
                    )

            def d0_half(rt, h):
                half_tile(d0T, d1T, rt, h, NCH_N, h8_0, sim_o, "sq0")

            def d1_half(rt, h):
                half_tile(d1T, d0T, rt, h, NCH_M, h8_1, None, "sq1")

            # phase A: first-half rowtiles x first-half columns of both
            # directions — depends only on the first-half prologues. The
            # second-half prologue work is interleaved into phase A's emission
            # so ACT/DVE drain it gradually behind the main-loop work.
            phase_a = NCH_N > 1 and NCH_M > 1 and HN == HM and RTN == RTM
            if phase_a:
                nA = HN
                nsq = max(1, nA // 2)  # iters for pro2 squares (on DVE)
                sq_per = -(-HN // nsq)
                nfl = nA - nsq - 1  # iters for fills, 4-tile groups
                for rt in range(nA):
                    d0_half(rt, 0)
                    d1_half(rt, 0)
                    if rt < nsq:
                        a = HN + rt * sq_per
                        b = min(RTN, a + sq_per)
                        if a < b:
                            pro_squares(d0, a, b, "0", force_dve=True)
                            pro_squares(d1, a, b, "1")
                    if rt == nsq:
                        pro_norm(HN, RTN, "0")
                        pro_norm(HM, RTM, "1")
                    if rt > nsq:
                        # alternate 4-tile fill groups between tensors
                        k = rt - nsq - 1
                        ngroups = -(-HN // 4)
                        if k < ngroups:
                            a = HN + k * 4
                            b = min(RTN, a + 4)
                            pro_fill(d0T, a, b, "0")
                        elif k - ngroups < ngroups:
                            a = HN + (k - ngroups) * 4
                            b = min(RTN, a + 4)
                            pro_fill(d1T, a, b, "1")
                # any fill groups that didn't fit inside phase A
                done = max(0, nA - nsq - 1)
                ngroups = -(-HN // 4)
                for k in range(done, 2 * ngroups):
                    if k < ngroups:
                        a, b = HN + k * 4, min(RTN, HN + k * 4 + 4)
                        pro_fill(d0T, a, b, "0")
                    else:
                        kk = k - ngroups
                        a, b = HN + kk * 4, min(RTN, HN + kk * 4 + 4)
                        pro_fill(d1T, a, b, "1")
            else:
                pro_squares(d0, HN, RTN, "0")
                pro_squares(d1, HM, RTM, "1")
                pro_norm(HN, RTN, "0")
                pro_norm(HM, RTM, "1")
                pro_fill(d0T, HN, RTN, "0")
                pro_fill(d1T, HM, RTM, "1")

            # phase B: everything else
            for rt in range(max(RTN, RTM)):
                if rt < RTN:
                    for h in range(NCH_N):
                        if rt < HN and h == 0 and phase_a:
                            continue
                        d0_half(rt, h)
                if rt < RTM:
                    for h in range(NCH_M):
                        if rt < HM and h == 0 and phase_a:
                            continue
                        d1_half(rt, h)

            nc.sync.dma_start(
                th0.ap().rearrange("a (t p k) -> p a t k", p=128, k=NCH_N * 8),
                h8_0[:].rearrange("p (t k) -> p t k", k=NCH_N * 8),
            )
            nc.sync.dma_start(
                th1.ap().rearrange("a (t p k) -> p a t k", p=128, k=NCH_M * 8),
                h8_1[:].rearrange("p (t k) -> p t k", k=NCH_M * 8),
            )

    nc.compile()
    return nc


def _get_nc(n=N, m=M, d=D):
    key = (n, m, d)
    if key not in _CACHE:
        _CACHE[key] = _build(n, m, d)
    return _CACHE[key]


def _find_nn_host(cand, sim_rows):
    """cand: [R, K] top candidates per row; sim_rows: callable i -> sim row.

    Returns pre-mutual matches [R] int32 (-1 or argmax index)."""
    r2 = RATIO_THRESH * RATIO_THRESH
    d2 = DIST_THRESH * DIST_THRESH
    part = np.partition(cand, cand.shape[1] - 2, axis=1)
    v1 = part[:, -1]
    v2 = part[:, -2]
    dist1 = 2.0 * (1.0 - v1)
    dist2 = 2.0 * (1.0 - v2)
    mask = (dist1 <= r2 * dist2) & (dist1 <= d2)
    out = np.full(cand.shape[0], -1, dtype=np.int32)
    for i in np.nonzero(mask)[0]:
        out[i] = int(np.argmax(sim_rows(int(i))))
    return out


def _mutual_check(m0, m1):
    i0 = np.arange(m0.shape[-1])[None, :]
    i1 = np.arange(m1.shape[-1])[None, :]
    loop0 = np.take_along_axis(m1, np.where(m0 > -1, m0, 0), axis=-1)
    loop1 = np.take_along_axis(m0, np.where(m1 > -1, m1, 0), axis=-1)
    m0n = np.where((m0 > -1) & (i0 == loop0), m0, -1)
    m1n = np.where((m1 > -1) & (i1 == loop1), m1, -1)
    return m0n, m1n


def _postprocess(res, n=N, m=M):
    half = min(HALF, m)
    nch_n, nch_m = m // half, n // half
    sim = np.stack([res.results[b]["sim"] for b in range(B)])
    m0 = np.empty((B, n), dtype=np.int32)
    m1 = np.empty((B, m), dtype=np.int32)
    for b in range(B):
        c0 = res.results[b]["top8h0"].reshape(n // 128, 128, nch_n * 8)
        c0 = c0.reshape(n, nch_n * 8)
        c1 = res.results[b]["top8h1"].reshape(m // 128, 128, nch_m * 8)
        c1 = c1.reshape(m, nch_m * 8)
        m0[b] = _find_nn_host(c0, lambda i: sim[b, i, :])
        m1[b] = _find_nn_host(c1, lambda j: sim[b, :, j])
    m0, m1 = _mutual_check(m0, m1)
    ms0 = (m0 > -1).astype(np.float32)
    ms1 = (m1 > -1).astype(np.float32)
    return (
        m0.astype(np.int32),
        m1.astype(np.int32),
        ms0,
        ms1,
        sim.astype(np.float32),
    )


def _execute(descriptors0, descriptors1, trace=False, trace_cores=None):
    from concourse.bass_utils import run_bass_kernel_spmd

    assert descriptors0.shape == (B, N, D) and descriptors1.shape == (B, M, D)
    nc = _get_nc()
    in_maps = [
        {
            "descriptors0": np.ascontiguousarray(descriptors0[b], dtype=np.float32),
            "descriptors1": np.ascontiguousarray(descriptors1[b], dtype=np.float32),
        }
        for b in range(B)
    ]
    res = run_bass_kernel_spmd(
        nc, in_maps, core_ids=list(range(B)), trace=trace, trace_cores=trace_cores
    )
    return _postprocess(res), res


def kernel(descriptors0: np.ndarray, descriptors1: np.ndarray):
    out, _ = _execute(descriptors0, descriptors1)
    return out
